# revision 12
# baseline (speedup 1.0000x reference)
"""Trainium2 Bass kernel for nn_Attention_Encode (B=4, N=2048, DIM=1024, H=16, DH=64).

v2 — batch x head-half hybrid sharding. Core c handles batch c//2 and the
8 heads (c%2)*8..(c%2)*8+7 (512 of the 1024 UT channels). Each core emits
4 bf16 pair-partial projections [N, C]; the host sums the 2x4 partials per
batch (the all-reduce of the row-sharded output projection).

Per-core structure:
  proj1   ztu_g = W_g @ ZT_b^T          -> ztuT [128(pair ch), 4 pairs, N]
  attn    per head pair, exploiting score-matrix symmetry (Q=K=V):
          for query block J only key strips kt >= 4J are computed (QK+exp);
          the strips kt >= 4J+4 are bounced to DRAM and read back through
          the DMA xbar TRANSPOSED, landing as ready-made AV operands for
          the skipped (upper) strips of later query blocks. Halves ACT exp.
          QK runs both heads concurrently via PE row tiling (two K=64
          matmuls at tile_position (0,0)/(64,0)). AV uses a [v|ones|v]
          stationary so softmax denominators ride in the matmul.
  proj2   per-pair partial = ssa_pair @ W_pair, streamed out in bf16.
"""
import sys

for _p in ('/opt/trn_rl_repo',):
    if _p not in sys.path:
        sys.path.insert(0, _p)

from contextlib import ExitStack

import numpy as np
import ml_dtypes

import concourse.bacc as bacc
import concourse.mybir as mybir
import concourse.tile as tile
from concourse.bass_utils import run_bass_kernel_spmd
from concourse.masks import make_identity

B, N, C = 4, 2048, 1024          # batch, seq, model dim
KP = 512                         # per-core UT channels (8 heads)
NPAIR = 4                        # head pairs per core
NQB = 512                        # query block
NKT = 128                        # key tile
NTB = N // NKT                   # 16 key strips
DH = 64
SCALE = DH ** -0.5               # 0.125
SYM = True                       # symmetric upper-strip fill via DMA transpose
BF = mybir.dt.bfloat16
F32 = mybir.dt.float32
F32R = mybir.dt.float32r

_CACHE = {}


def _build_kernel():
    nc = bacc.Bacc("TRN2", target_bir_lowering=False, debug=False)
    ztt = nc.dram_tensor("ztt", [C, N], BF, kind="ExternalInput").ap()
    wgt = nc.dram_tensor("wgt", [C, KP], BF, kind="ExternalInput").ap()   # W_g^T
    wg = nc.dram_tensor("wg", [KP, C], BF, kind="ExternalInput").ap()     # W_g
    outp = nc.dram_tensor("outp", [NPAIR, N, C], BF, kind="ExternalOutput").ap()

    with tile.TileContext(nc) as tc, ExitStack() as ctx:
        _body(ctx, tc, ztt, wgt, wg, outp)
    nc.compile()
    return nc


def _body(ctx, tc, ztt, wgt, wg, outp):
    nc = tc.nc
    singles = ctx.enter_context(tc.tile_pool(name="singles", bufs=1))
    sc_pool = ctx.enter_context(tc.tile_pool(name="sc", bufs=1, space="PSUM"))
    av_pool = ctx.enter_context(tc.tile_pool(name="av", bufs=4, space="PSUM"))
    p2_pool = ctx.enter_context(tc.tile_pool(name="p2", bufs=2, space="PSUM"))
    tt_pool = ctx.enter_context(tc.tile_pool(name="tt", bufs=26))
    sn_pool = ctx.enter_context(tc.tile_pool(name="sn", bufs=3))
    rc_pool = ctx.enter_context(tc.tile_pool(name="rc", bufs=2))
    p2s_pool = ctx.enter_context(tc.tile_pool(name="p2s", bufs=4))
    bn_pool = ctx.enter_context(tc.tile_pool(name="bn", bufs=12, space="DRAM"))

    # ---- persistent SBUF ----
    zin = singles.tile([128, 8, N], BF)                # ZT^T chunks [c-in-tile, ci, n]
    ztuT = singles.tile([128, NPAIR, N], BF)           # [pair-ch, pair, n]
    ztuN = singles.tile([128, NTB, NPAIR, 132], BF)    # [n-in-tile, kt, pair, vA|1|vB]
    wgt_sb = singles.tile([128, 8, NPAIR, 128], BF)
    wg_sb = singles.tile([128, NPAIR, C], BF)
    ident = singles.tile([128, 128], BF)
    make_identity(nc, ident)
    self_f = singles.tile([128, 128], F32)
    nc.vector.memset(self_f, 0.0)
    nc.vector.memset(self_f[0:1, 0:64], 1.0)
    nc.vector.memset(self_f[32:33, 64:128], 1.0)
    sel = singles.tile([128, 128], F32R)               # den -> per-head row broadcast
    nc.vector.tensor_copy(out=sel, in_=self_f)
    dn = singles.tile([128, NQB], F32R)                # dens: head A row 0, B row 32
    nc.vector.memset(dn[:].bitcast(F32), 0.0)
    # exp tiles, indexed by global key strip: [p, kt, head, q]
    ex_all = singles.tile([128, NTB, 2, NQB], BF)
    nc.gpsimd.memset(ztuN, 0.0)
    nc.gpsimd.memset(ztuN[:, :, :, 64:65], 1.0)
    nc.gpsimd.memset(ztuN[:, :, :, 129:130], 1.0)

    # ---- input DMAs (ACT ring) ----
    nc.scalar.dma_start(
        out=wgt_sb, in_=wgt.rearrange("(ci p) (t x) -> p ci t x", p=128, x=128))
    nc.scalar.dma_start(out=wg_sb, in_=wg.rearrange("(P p) c -> p P c", p=128))

    # PE warm-up spin over the load window (HAM clock gate)
    warm = p2_pool.tile([128, NQB], F32, tag="p2", name="warm")
    for _ in range(64):
        nc.tensor.matmul(warm[:, 0:32], lhsT=ident, rhs=ident[:, 0:32],
                         start=True, stop=True)
    del warm

    for jn in range(4):
        for ci in range(8):
            nc.scalar.dma_start(
                out=zin[:, ci, jn * NQB:(jn + 1) * NQB],
                in_=ztt[ci * 128:(ci + 1) * 128, jn * NQB:(jn + 1) * NQB])

    # ---- proj1 + V transposes ----
    for t in range(NPAIR):
        for jn in range(4):
            p1 = av_pool.tile([128, NQB], F32, tag="av", name="p1")
            for ci in range(8):
                nc.tensor.matmul(
                    p1, lhsT=wgt_sb[:, ci, t, :],
                    rhs=zin[:, ci, jn * NQB:(jn + 1) * NQB],
                    start=(ci == 0), stop=(ci == 7))
            nc.vector.tensor_copy(
                out=ztuT[:, t, jn * NQB:(jn + 1) * NQB], in_=p1)
        for kt in range(NTB):
            pt = av_pool.tile([128, NQB], BF, tag="av", name="pt")
            nc.tensor.transpose(
                pt[:, 0:128], ztuT[:, t, kt * NKT:(kt + 1) * NKT], ident)
            nc.vector.tensor_copy(
                out=ztuN[:, kt, t, 0:130].rearrange("p (b v) -> p b v", b=2)[:, :, 0:64],
                in_=pt[:, 0:128].rearrange("p (b v) -> p b v", b=2))

    # ---- attention + proj2, per head pair ----
    # av row layout (both heads): rows 0:64 = v, row 64 = den (ones column)
    hsl = ((0, 65), (65, 130))

    bn_tiles = {}      # (h, J) -> bounce DRAM tile of this pair
    tt_tiles = {}      # (a, h) -> transposed strip for the UPCOMING qb

    def emit_reads(P, J):
        # transposed reads for qb J's fills (a < 4J), ascending a
        for a in range(4 * J):
            js = a >> 2
            s0 = 4 * (J - js - 1)
            c0 = (a & 3) * NKT
            for h in range(2):
                tt = tt_pool.tile([128, NQB], BF, tag="tt", name=f"tt{h}")
                nc.sync.dma_start_transpose(
                    out=tt,
                    in_=bn_tiles[(h, js)][s0:s0 + 4, :, c0:c0 + NKT]
                    .rearrange("s j q -> (s j) q"))
                tt_tiles[(a, h)] = tt

    def do_qb(P, J):
        q0 = J * NQB
        nstr = NTB - 4 * J if SYM else NTB
        b0 = NTB - nstr
        avs = [av_pool.tile([128, NQB], F32, tag="av", name=f"av{h}")
               for h in range(2)]
        ncontrib = NTB  # per-head accumulation steps into av
        nem = [0, 0]    # emitted per head

        fills = [(a, h) for a in range(b0) for h in range(2)] if SYM else []
        nfe = 0         # fills emitted

        def av_mm(h, lkt, rhs):
            nc.tensor.matmul(
                avs[h][0:65, :], lhsT=ztuN[:, lkt, P, hsl[h][0]:hsl[h][1]],
                rhs=rhs,
                start=(nem[h] == 0), stop=(nem[h] == ncontrib - 1))
            nem[h] += 1

        def emit_fills(upto):
            nonlocal nfe
            while nfe < upto:
                a, h = fills[nfe]
                av_mm(h, a, tt_tiles.pop((a, h)))
                nfe += 1

        for g in range(nstr):
            b = b0 + g
            sc = sc_pool.tile([128, 2 * NQB], F32, tag="sc")
            for h in range(2):
                r0 = 64 * h
                nc.tensor.matmul(
                    sc[:, h * NQB:(h + 1) * NQB],
                    lhsT=ztuT[r0:r0 + 64, P, b * NKT:(b + 1) * NKT],
                    rhs=ztuT[r0:r0 + 64, P, q0:q0 + NQB],
                    start=True, stop=True, tile_position=(r0, 0))
            nc.scalar.activation(
                out=ex_all[:, b, :, :].rearrange("p h q -> p (h q)"), in_=sc,
                func=mybir.ActivationFunctionType.Exp, scale=SCALE)
            for h in range(2):
                av_mm(h, b, ex_all[:, b, h, :])
            if fills and g >= 2:
                emit_fills(len(fills) if g == nstr - 1
                           else -(-len(fills) * (g - 1) // (nstr - 2)))
            if SYM and J < 3:
                # bounce computed strips for later qbs (ACT ring), in two
                # chunks so early readers aren't gated on the full write
                if b == 11 and b0 + 4 <= 11:
                    for h in range(2):
                        bn = bn_pool.tile([12, 128, NQB], BF, tag=f"bn{h}{J}",
                                          name=f"bn{h}{J}")
                        bn_tiles[(h, J)] = bn
                        nc.scalar.dma_start(
                            out=bn[0:8 - 4 * J].rearrange("s j q -> j s q"),
                            in_=ex_all[:, b0 + 4:12, h, :])
                if b == 15:
                    for h in range(2):
                        if (h, J) not in bn_tiles:
                            bn = bn_pool.tile([12, 128, NQB], BF,
                                              tag=f"bn{h}{J}", name=f"bn{h}{J}")
                            bn_tiles[(h, J)] = bn
                        nc.scalar.dma_start(
                            out=bn_tiles[(h, J)][8 - 4 * J:12 - 4 * J]
                            .rearrange("s j q -> j s q"),
                            in_=ex_all[:, 12:16, h, :])
        if fills:
            emit_fills(len(fills))
        if SYM and J < 3:
            emit_reads(P, J + 1)

        # ---- softmax normalize ----
        nc.vector.tensor_copy(out=dn[0:1, :], in_=avs[0][64:65, :])
        nc.vector.tensor_copy(out=dn[32:33, :], in_=avs[1][64:65, :])
        bc = p2_pool.tile([128, NQB], F32, tag="p2", name="bc")
        nc.tensor.matmul(bc, lhsT=sel, rhs=dn, start=True, stop=True)
        rcv = rc_pool.tile([128, NQB], F32)
        nc.vector.reciprocal_approx_fast(out=rcv, in_=bc)
        sn = sn_pool.tile([128, NQB], BF)
        nc.vector.tensor_tensor(
            out=sn[0:64, :], in0=avs[0][0:DH, :], in1=rcv[0:64, :],
            op=mybir.AluOpType.mult)
        nc.vector.tensor_tensor(
            out=sn[64:128, :], in0=avs[1][0:DH, :], in1=rcv[64:128, :],
            op=mybir.AluOpType.mult)

        # ---- proj2 partial for this pair / query block ----
        for tq in range(4):
            for ch in range(2):
                p2 = p2_pool.tile([128, NQB], F32, tag="p2", name="p2")
                nc.tensor.matmul(
                    p2, lhsT=sn[:, tq * 128:(tq + 1) * 128],
                    rhs=wg_sb[:, P, ch * NQB:(ch + 1) * NQB],
                    start=True, stop=True)
                p2s = p2s_pool.tile([128, NQB], BF, tag="p2s")
                nc.vector.tensor_copy(out=p2s, in_=p2)
                r0 = q0 + tq * 128
                nc.gpsimd.dma_start(
                    out=outp[P, r0:r0 + 128, ch * NQB:(ch + 1) * NQB], in_=p2s)

    for P in range(NPAIR):
        for J in range(4):
            do_qb(P, J)
        bn_tiles.clear()


def _get_nc():
    if "nc" not in _CACHE:
        _CACHE["nc"] = _build_kernel()
    return _CACHE["nc"]


def make_in_maps(ZT, W):
    ZT = np.asarray(ZT, dtype=np.float32)
    W = np.asarray(W, dtype=np.float32)
    ztt = [np.ascontiguousarray(ZT[b].T).astype(ml_dtypes.bfloat16)
           for b in range(B)]
    wghalf = []
    for g in range(2):
        wgf = W[g * KP:(g + 1) * KP, :]
        wghalf.append((np.ascontiguousarray(wgf.T).astype(ml_dtypes.bfloat16),
                       np.ascontiguousarray(wgf).astype(ml_dtypes.bfloat16)))
    in_maps = []
    for c in range(8):
        b, g = c // 2, c % 2
        in_maps.append({"ztt": ztt[b], "wgt": wghalf[g][0], "wg": wghalf[g][1]})
    return in_maps


def kernel(ZT: np.ndarray, W: np.ndarray) -> np.ndarray:
    nc = _get_nc()
    res = run_bass_kernel_spmd(nc, make_in_maps(ZT, W), core_ids=list(range(8)))
    out = np.zeros((B, N, C), dtype=np.float32)
    for c in range(8):
        out[c // 2] += res.results[c]["outp"].astype(np.float32).sum(axis=0)
    return out


if __name__ == "__main__":
    rng = np.random.default_rng(0)
    zt = rng.standard_normal((B, N, C), dtype=np.float32)
    w = rng.standard_normal((C, C), dtype=np.float32) * C ** -0.5
    o = kernel(zt, w)
    print("out", o.shape, o.dtype, float(np.abs(o).mean()))


# revision 20
# speedup vs baseline: 1.1464x; 1.1464x over previous
"""Trainium2 Bass kernel for nn_Attention_Encode (B=4, N=2048, DIM=1024, H=16, DH=64).

v2 — batch x head-half hybrid sharding. Core c handles batch c//2 and the
8 heads (c%2)*8..(c%2)*8+7 (512 of the 1024 UT channels). Each core emits
4 bf16 pair-partial projections [N, C]; the host sums the 2x4 partials per
batch (the all-reduce of the row-sharded output projection).

Per-core structure:
  proj1   ztu_g = W_g @ ZT_b^T          -> ztuT [128(pair ch), 4 pairs, N]
  attn    per head pair, exploiting score-matrix symmetry (Q=K=V):
          for query block J only key strips kt >= 4J are computed (QK+exp);
          the strips kt >= 4J+4 are bounced to DRAM and read back through
          the DMA xbar TRANSPOSED, landing as ready-made AV operands for
          the skipped (upper) strips of later query blocks. Halves ACT exp.
          QK runs both heads concurrently via PE row tiling (two K=64
          matmuls at tile_position (0,0)/(64,0)). AV uses a [v|ones|v]
          stationary so softmax denominators ride in the matmul.
  proj2   per-pair partial = ssa_pair @ W_pair, streamed out in bf16.
"""
import sys

for _p in ('/opt/trn_rl_repo',):
    if _p not in sys.path:
        sys.path.insert(0, _p)

from contextlib import ExitStack

import numpy as np
import ml_dtypes

import concourse.bacc as bacc
import concourse.mybir as mybir
import concourse.tile as tile
from concourse.bass_utils import run_bass_kernel_spmd
from concourse.masks import make_identity

B, N, C = 4, 2048, 1024          # batch, seq, model dim
KP = 512                         # per-core UT channels (8 heads)
NPAIR = 4                        # head pairs per core
NQB = 512                        # query block
NKT = 128                        # key tile
NTB = N // NKT                   # 16 key strips
DH = 64
SCALE = DH ** -0.5               # 0.125
SYM = True                       # symmetric upper-strip fill via DMA transpose
BF = mybir.dt.bfloat16
F32 = mybir.dt.float32
F32R = mybir.dt.float32r

_CACHE = {}


def _build_kernel():
    nc = bacc.Bacc("TRN2", target_bir_lowering=False, debug=False)
    ztt = nc.dram_tensor("ztt", [C, N], BF, kind="ExternalInput").ap()
    wgt = nc.dram_tensor("wgt", [C, KP], BF, kind="ExternalInput").ap()   # W_g^T
    wg = nc.dram_tensor("wg", [KP, C], BF, kind="ExternalInput").ap()     # W_g
    outp = nc.dram_tensor("outp", [NPAIR, N, C], BF, kind="ExternalOutput").ap()

    with tile.TileContext(nc) as tc, ExitStack() as ctx:
        _body(ctx, tc, ztt, wgt, wg, outp)
    nc.compile()
    return nc


def _body(ctx, tc, ztt, wgt, wg, outp):
    nc = tc.nc
    singles = ctx.enter_context(tc.tile_pool(name="singles", bufs=1))
    sc_pool = ctx.enter_context(tc.tile_pool(name="sc", bufs=2, space="PSUM"))
    av_pool = ctx.enter_context(tc.tile_pool(name="av", bufs=4, space="PSUM"))
    tt_pool = ctx.enter_context(tc.tile_pool(name="tt", bufs=26))
    sn_pool = ctx.enter_context(tc.tile_pool(name="sn", bufs=3))
    rc_pool = ctx.enter_context(tc.tile_pool(name="rc", bufs=2))
    p2s_pool = ctx.enter_context(tc.tile_pool(name="p2s", bufs=4))
    bn_pool = ctx.enter_context(tc.tile_pool(name="bn", bufs=12, space="DRAM"))

    # ---- persistent SBUF ----
    zin = singles.tile([128, 8, N], BF)                # ZT^T chunks [c-in-tile, ci, n]
    ztuT = singles.tile([128, NPAIR, N], BF)           # [pair-ch, pair, n]
    ztuN = singles.tile([128, NTB, NPAIR, 132], BF)    # [n-in-tile, kt, pair, vA|1|vB]
    wgt_sb = singles.tile([128, 8, NPAIR, 128], BF)
    wg_sb = singles.tile([128, NPAIR, C], BF)
    ident = singles.tile([128, 128], BF)
    make_identity(nc, ident)
    self_f = singles.tile([128, 128], F32)
    nc.vector.memset(self_f, 0.0)
    nc.vector.memset(self_f[0:1, 0:64], 1.0)
    nc.vector.memset(self_f[32:33, 64:128], 1.0)
    sel = singles.tile([128, 128], F32R)               # den -> per-head row broadcast
    nc.vector.tensor_copy(out=sel, in_=self_f)
    dn = singles.tile([128, NQB], F32R)                # dens: head A row 0, B row 32
    nc.vector.memset(dn[:].bitcast(F32), 0.0)
    # exp tiles, indexed by global key strip: [p, kt, head, q]
    ex_all = singles.tile([128, NTB, 2, NQB], BF)
    nc.gpsimd.memset(ztuN, 0.0)
    nc.gpsimd.memset(ztuN[:, :, :, 64:65], 1.0)
    nc.gpsimd.memset(ztuN[:, :, :, 129:130], 1.0)

    # ---- input DMAs (ACT ring) ----
    nc.scalar.dma_start(
        out=wgt_sb, in_=wgt.rearrange("(ci p) (t x) -> p ci t x", p=128, x=128))
    nc.scalar.dma_start(out=wg_sb, in_=wg.rearrange("(P p) c -> p P c", p=128))

    # PE warm-up spin over the load window (HAM clock gate)
    warm = av_pool.tile([128, NQB], F32, tag="av", name="warm")
    for _ in range(64):
        nc.tensor.matmul(warm[:, 0:32], lhsT=ident, rhs=ident[:, 0:32],
                         start=True, stop=True)
    del warm

    for jn in range(4):
        for ci in range(8):
            nc.sync.dma_start(
                out=zin[:, ci, jn * NQB:(jn + 1) * NQB],
                in_=ztt[ci * 128:(ci + 1) * 128, jn * NQB:(jn + 1) * NQB])

    # ---- proj1 + V transposes ----
    for t in range(NPAIR):
        for jn in range(4):
            p1 = av_pool.tile([128, NQB], F32, tag="av", name="p1")
            for ci in range(8):
                nc.tensor.matmul(
                    p1, lhsT=wgt_sb[:, ci, t, :],
                    rhs=zin[:, ci, jn * NQB:(jn + 1) * NQB],
                    start=(ci == 0), stop=(ci == 7))
            nc.vector.tensor_copy(
                out=ztuT[:, t, jn * NQB:(jn + 1) * NQB], in_=p1)
        for kt in range(NTB):
            pt = av_pool.tile([128, NQB], BF, tag="av", name="pt")
            nc.tensor.transpose(
                pt[:, 0:128], ztuT[:, t, kt * NKT:(kt + 1) * NKT], ident)
            nc.vector.tensor_copy(
                out=ztuN[:, kt, t, 0:130].rearrange("p (b v) -> p b v", b=2)[:, :, 0:64],
                in_=pt[:, 0:128].rearrange("p (b v) -> p b v", b=2))

    # ---- attention + proj2, per head pair ----
    # av row layout (both heads): rows 0:64 = v, row 64 = den (ones column)
    hsl = ((0, 65), (65, 130))

    bn_tiles = {}      # (h, J) -> bounce DRAM tile of this pair
    tt_tiles = {}      # (a, h) -> transposed strip for the UPCOMING qb
    pending = []       # deferred PE-containing work items, drained 1/strip

    def drain(n=1):
        for _ in range(n):
            if pending:
                pending.pop(0)()

    def emit_reads(P, J):
        # transposed reads for qb J's fills (a < 4J), ascending a.
        # Sources with Jsrc < J-1 are long since written -> ACT ring (its
        # queue must never block); Jsrc == J-1 just got written -> SYNC ring.
        for a in range(4 * J):
            js = a >> 2
            s0 = 4 * (J - js - 1)
            c0 = (a & 3) * NKT
            eng = nc.scalar if js < J - 1 else nc.sync
            for h in range(2):
                tt = tt_pool.tile([128, NQB], BF, tag="tt", name=f"tt{h}")
                eng.dma_start_transpose(
                    out=tt,
                    in_=bn_tiles[(h, js)][s0:s0 + 4, :, c0:c0 + NKT]
                    .rearrange("s j q -> (s j) q"))
                tt_tiles[(a, h)] = tt

    def do_qb(P, J):
        q0 = J * NQB
        nstr = NTB - 4 * J if SYM else NTB
        b0 = NTB - nstr
        avs = [av_pool.tile([128, NQB], F32, tag="av", name=f"av{h}")
               for h in range(2)]
        ncontrib = NTB  # per-head accumulation steps into av
        nem = [0, 0]    # emitted per head

        fills = [(a, h) for a in range(b0) for h in range(2)] if SYM else []
        nfe = 0         # fills emitted

        def av_mm(h, lkt, rhs):
            nc.tensor.matmul(
                avs[h][0:65, :], lhsT=ztuN[:, lkt, P, hsl[h][0]:hsl[h][1]],
                rhs=rhs,
                start=(nem[h] == 0), stop=(nem[h] == ncontrib - 1))
            nem[h] += 1

        def emit_fills(upto):
            nonlocal nfe
            while nfe < upto:
                a, h = fills[nfe]
                av_mm(h, a, tt_tiles.pop((a, h)))
                nfe += 1

        # AV for strip g-1 is emitted after strip g's QK so the in-order PE
        # queue never waits on exp; deferred items drain in pairs after odd
        # strips to keep QK's sc double-buffer slot parity.
        for g in range(nstr):
            b = b0 + g
            sc = sc_pool.tile([128, 2 * NQB], F32, tag="sc")
            for h in range(2):
                r0 = 64 * h
                nc.tensor.matmul(
                    sc[:, h * NQB:(h + 1) * NQB],
                    lhsT=ztuT[r0:r0 + 64, P, b * NKT:(b + 1) * NKT],
                    rhs=ztuT[r0:r0 + 64, P, q0:q0 + NQB],
                    start=True, stop=True, tile_position=(r0, 0))
            nc.scalar.activation(
                out=ex_all[:, b, :, :].rearrange("p h q -> p (h q)"), in_=sc,
                func=mybir.ActivationFunctionType.Exp, scale=SCALE)
            if g >= 1:
                for h in range(2):
                    av_mm(h, b - 1, ex_all[:, b - 1, h, :])
            if g % 2 == 1:
                drain(2)
            if fills and g >= 2:
                emit_fills(len(fills) if g == nstr - 1
                           else -(-len(fills) * (g - 1) // (nstr - 2)))
            if SYM and J < 3:
                # bounce computed strips for later qbs (ACT ring), in two
                # chunks so early readers aren't gated on the full write
                if b == 11 and b0 + 4 <= 11:
                    for h in range(2):
                        bn = bn_pool.tile([12, 128, NQB], BF, tag=f"bn{h}{J}",
                                          name=f"bn{h}{J}")
                        bn_tiles[(h, J)] = bn
                        nc.scalar.dma_start(
                            out=bn[0:8 - 4 * J].rearrange("s j q -> j s q"),
                            in_=ex_all[:, b0 + 4:12, h, :])
                if b == 15:
                    for h in range(2):
                        if (h, J) not in bn_tiles:
                            bn = bn_pool.tile([12, 128, NQB], BF,
                                              tag=f"bn{h}{J}", name=f"bn{h}{J}")
                            bn_tiles[(h, J)] = bn
                        nc.scalar.dma_start(
                            out=bn_tiles[(h, J)][8 - 4 * J:12 - 4 * J]
                            .rearrange("s j q -> j s q"),
                            in_=ex_all[:, 12:16, h, :])
        for h in range(2):
            av_mm(h, NTB - 1, ex_all[:, NTB - 1, h, :])
        if fills:
            emit_fills(len(fills))
        if SYM and J < 3:
            emit_reads(P, J + 1)

        # ---- softmax normalize + proj2, deferred into the next qb's
        # instruction stream so the in-order PE queue never waits on the
        # DVE normalization chain ----
        nc.vector.tensor_copy(out=dn[0:1, :], in_=avs[0][64:65, :])
        nc.vector.tensor_copy(out=dn[32:33, :], in_=avs[1][64:65, :])
        sn = sn_pool.tile([128, NQB], BF)

        def norm_item(avs=avs, sn=sn):
            bc = sc_pool.tile([128, 2 * NQB], F32, tag="sc", name="bc")
            bcv = bc[:, 0:NQB]
            nc.tensor.matmul(bcv, lhsT=sel, rhs=dn, start=True, stop=True)
            rcv = rc_pool.tile([128, NQB], F32)
            nc.vector.reciprocal_approx_fast(out=rcv, in_=bcv)
            nc.vector.tensor_tensor(
                out=sn[0:64, :], in0=avs[0][0:DH, :], in1=rcv[0:64, :],
                op=mybir.AluOpType.mult)
            nc.vector.tensor_tensor(
                out=sn[64:128, :], in0=avs[1][0:DH, :], in1=rcv[64:128, :],
                op=mybir.AluOpType.mult)

        def p2_item(tq, ch, P=P, q0=q0, sn=sn):
            p2 = sc_pool.tile([128, 2 * NQB], F32, tag="sc", name="p2")
            p2v = p2[:, 0:NQB]
            nc.tensor.matmul(
                p2v, lhsT=sn[:, tq * 128:(tq + 1) * 128],
                rhs=wg_sb[:, P, ch * NQB:(ch + 1) * NQB],
                start=True, stop=True)
            p2s = p2s_pool.tile([128, NQB], BF, tag="p2s")
            nc.vector.tensor_copy(out=p2s, in_=p2v)
            r0 = q0 + tq * 128
            nc.gpsimd.dma_start(
                out=outp[P, r0:r0 + 128, ch * NQB:(ch + 1) * NQB], in_=p2s)

        pending.append(norm_item)
        for tq in range(4):
            for ch in range(2):
                pending.append(lambda tq=tq, ch=ch: p2_item(tq, ch))

    for P in range(NPAIR):
        for J in range(4):
            do_qb(P, J)
        bn_tiles.clear()
    drain(len(pending))


def _get_nc():
    if "nc" not in _CACHE:
        _CACHE["nc"] = _build_kernel()
    return _CACHE["nc"]


def make_in_maps(ZT, W):
    ZT = np.asarray(ZT, dtype=np.float32)
    W = np.asarray(W, dtype=np.float32)
    ztt = [np.ascontiguousarray(ZT[b].T).astype(ml_dtypes.bfloat16)
           for b in range(B)]
    wghalf = []
    for g in range(2):
        wgf = W[g * KP:(g + 1) * KP, :]
        wghalf.append((np.ascontiguousarray(wgf.T).astype(ml_dtypes.bfloat16),
                       np.ascontiguousarray(wgf).astype(ml_dtypes.bfloat16)))
    in_maps = []
    for c in range(8):
        b, g = c // 2, c % 2
        in_maps.append({"ztt": ztt[b], "wgt": wghalf[g][0], "wg": wghalf[g][1]})
    return in_maps


def kernel(ZT: np.ndarray, W: np.ndarray) -> np.ndarray:
    nc = _get_nc()
    res = run_bass_kernel_spmd(nc, make_in_maps(ZT, W), core_ids=list(range(8)))
    out = np.zeros((B, N, C), dtype=np.float32)
    for c in range(8):
        out[c // 2] += res.results[c]["outp"].astype(np.float32).sum(axis=0)
    return out


if __name__ == "__main__":
    rng = np.random.default_rng(0)
    zt = rng.standard_normal((B, N, C), dtype=np.float32)
    w = rng.standard_normal((C, C), dtype=np.float32) * C ** -0.5
    o = kernel(zt, w)
    print("out", o.shape, o.dtype, float(np.abs(o).mean()))


# revision 25
# speedup vs baseline: 1.2705x; 1.1083x over previous
"""Trainium2 Bass kernel for nn_Attention_Encode (B=4, N=2048, DIM=1024, H=16, DH=64).

v2 — batch x head-half hybrid sharding. Core c handles batch c//2 and the
8 heads (c%2)*8..(c%2)*8+7 (512 of the 1024 UT channels). Each core emits
4 bf16 pair-partial projections [N, C]; the host sums the 2x4 partials per
batch (the all-reduce of the row-sharded output projection).

Per-core structure:
  proj1   ztu_g = W_g @ ZT_b^T          -> ztuT [128(pair ch), 4 pairs, N]
  attn    per head pair, exploiting score-matrix symmetry (Q=K=V):
          for query block J only key strips kt >= 4J are computed (QK+exp);
          the strips kt >= 4J+4 are bounced to DRAM and read back through
          the DMA xbar TRANSPOSED, landing as ready-made AV operands for
          the skipped (upper) strips of later query blocks. Halves ACT exp.
          QK runs both heads concurrently via PE row tiling (two K=64
          matmuls at tile_position (0,0)/(64,0)). AV uses a [v|ones|v]
          stationary so softmax denominators ride in the matmul.
  proj2   per-pair partial = ssa_pair @ W_pair, streamed out in bf16.
"""
import sys

for _p in ('/opt/trn_rl_repo',):
    if _p not in sys.path:
        sys.path.insert(0, _p)

from contextlib import ExitStack

import numpy as np
import ml_dtypes

import concourse.bacc as bacc
import concourse.mybir as mybir
import concourse.tile as tile
from concourse.bass_utils import run_bass_kernel_spmd
from concourse.masks import make_identity

B, N, C = 4, 2048, 1024          # batch, seq, model dim
KP = 512                         # per-core UT channels (8 heads)
NPAIR = 4                        # head pairs per core
NQB = 512                        # query block
NKT = 128                        # key tile
NTB = N // NKT                   # 16 key strips
DH = 64
SCALE = DH ** -0.5               # 0.125
SYM = True                       # symmetric upper-strip fill via DMA transpose
BF = mybir.dt.bfloat16
F32 = mybir.dt.float32
F32R = mybir.dt.float32r

_CACHE = {}


def _build_kernel():
    nc = bacc.Bacc("TRN2", target_bir_lowering=False, debug=False)
    ztt = nc.dram_tensor("ztt", [C, N], BF, kind="ExternalInput").ap()
    wgt = nc.dram_tensor("wgt", [C, KP], BF, kind="ExternalInput").ap()   # W_g^T
    wg = nc.dram_tensor("wg", [KP, C], BF, kind="ExternalInput").ap()     # W_g
    outp = nc.dram_tensor("outp", [NPAIR, N, C], BF, kind="ExternalOutput").ap()

    with tile.TileContext(nc) as tc, ExitStack() as ctx:
        _body(ctx, tc, ztt, wgt, wg, outp)
    nc.compile()
    return nc


def _body(ctx, tc, ztt, wgt, wg, outp):
    nc = tc.nc
    singles = ctx.enter_context(tc.tile_pool(name="singles", bufs=1))
    sc_pool = ctx.enter_context(tc.tile_pool(name="sc", bufs=2, space="PSUM"))
    av_pool = ctx.enter_context(tc.tile_pool(name="av", bufs=4, space="PSUM"))
    tt_pool = ctx.enter_context(tc.tile_pool(name="tt", bufs=9))
    sn_pool = ctx.enter_context(tc.tile_pool(name="sn", bufs=3))
    rc_pool = ctx.enter_context(tc.tile_pool(name="rc", bufs=2))
    p2s_pool = ctx.enter_context(tc.tile_pool(name="p2s", bufs=4))
    bn_pool = ctx.enter_context(tc.tile_pool(name="bn", bufs=12, space="DRAM"))

    # ---- persistent SBUF ----
    zin = singles.tile([128, 8, N], BF)                # ZT^T chunks [c-in-tile, ci, n]
    ztuT = singles.tile([128, NPAIR, N], BF)           # [pair-ch, pair, n]
    ztuN = singles.tile([128, NTB, NPAIR, 132], BF)    # [n-in-tile, kt, pair, vA|1|vB]
    wgt_sb = singles.tile([128, 8, NPAIR, 128], BF)
    wg_sb = singles.tile([128, NPAIR, C], BF)
    ident = singles.tile([128, 128], BF)
    make_identity(nc, ident)
    self_f = singles.tile([128, 128], F32)
    nc.vector.memset(self_f, 0.0)
    nc.vector.memset(self_f[0:1, 0:64], 1.0)
    nc.vector.memset(self_f[32:33, 64:128], 1.0)
    sel = singles.tile([128, 128], F32R)               # den -> per-head row broadcast
    nc.vector.tensor_copy(out=sel, in_=self_f)
    dn = singles.tile([128, NQB], F32R)                # dens: head A row 0, B row 32
    nc.vector.memset(dn[:].bitcast(F32), 0.0)
    # exp tiles, indexed by global key strip: [p, kt, head, q]
    ex_all = singles.tile([128, NTB, 2, NQB], BF)
    nc.gpsimd.memset(ztuN, 0.0)
    nc.gpsimd.memset(ztuN[:, :, :, 64:65], 1.0)
    nc.gpsimd.memset(ztuN[:, :, :, 129:130], 1.0)

    # ---- input DMAs (ACT ring) ----
    nc.scalar.dma_start(
        out=wgt_sb, in_=wgt.rearrange("(ci p) (t x) -> p ci t x", p=128, x=128))
    nc.scalar.dma_start(out=wg_sb, in_=wg.rearrange("(P p) c -> p P c", p=128))

    # PE warm-up spin over the load window (HAM clock gate)
    warm = av_pool.tile([128, NQB], F32, tag="av", name="warm")
    for _ in range(64):
        nc.tensor.matmul(warm[:, 0:32], lhsT=ident, rhs=ident[:, 0:32],
                         start=True, stop=True)
    del warm

    for jn in range(4):
        for ci in range(8):
            nc.sync.dma_start(
                out=zin[:, ci, jn * NQB:(jn + 1) * NQB],
                in_=ztt[ci * 128:(ci + 1) * 128, jn * NQB:(jn + 1) * NQB])

    # ---- proj1 + V transposes ----
    for t in range(NPAIR):
        for jn in range(4):
            p1 = av_pool.tile([128, NQB], F32, tag="av", name="p1")
            for ci in range(8):
                nc.tensor.matmul(
                    p1, lhsT=wgt_sb[:, ci, t, :],
                    rhs=zin[:, ci, jn * NQB:(jn + 1) * NQB],
                    start=(ci == 0), stop=(ci == 7))
            nc.vector.tensor_copy(
                out=ztuT[:, t, jn * NQB:(jn + 1) * NQB], in_=p1)
        for kt in range(NTB):
            pt = av_pool.tile([128, NQB], BF, tag="av", name="pt")
            nc.tensor.transpose(
                pt[:, 0:128], ztuT[:, t, kt * NKT:(kt + 1) * NKT], ident)
            nc.vector.tensor_copy(
                out=ztuN[:, kt, t, 0:130].rearrange("p (b v) -> p b v", b=2)[:, :, 0:64],
                in_=pt[:, 0:128].rearrange("p (b v) -> p b v", b=2))

    # ---- attention + proj2, per head pair ----
    # av row layout (both heads): rows 0:64 = v, row 64 = den (ones column)
    hsl = ((0, 65), (65, 130))

    bn_tiles = {}      # (h, J) -> bounce DRAM tile of this pair
    tt_tiles = {}      # (a, h) -> transposed strip for the UPCOMING qb
    pending = []       # deferred PE-containing work items, drained 1/strip

    def drain(n=1):
        for _ in range(n):
            if pending:
                pending.pop(0)()

    def emit_reads(P, J):
        # One merged transposed read per (head, subcol a of qb J): covers
        # ALL later query blocks' fills for that a (sliced at consume time).
        nshared = 12 - 4 * J            # strips in bounce(J)
        for a in range(4 * J, 4 * J + 4):
            c0 = (a & 3) * NKT
            for h in range(2):
                tt = tt_pool.tile([128, nshared * NKT], BF, tag=f"tt{J}",
                                  name=f"tt{h}")
                nc.sync.dma_start_transpose(
                    out=tt,
                    in_=bn_tiles[(h, J)][0:nshared, :, c0:c0 + NKT]
                    .rearrange("s j q -> (s j) q"))
                tt_tiles[(a, h)] = tt

    def do_qb(P, J):
        q0 = J * NQB
        nstr = NTB - 4 * J if SYM else NTB
        b0 = NTB - nstr
        avs = [av_pool.tile([128, NQB], F32, tag="av", name=f"av{h}")
               for h in range(2)]
        ncontrib = NTB  # per-head accumulation steps into av
        nem = [0, 0]    # emitted per head

        fills = [(a, h) for a in range(b0) for h in range(2)] if SYM else []
        nfe = 0         # fills emitted

        def av_mm(h, lkt, rhs):
            nc.tensor.matmul(
                avs[h][0:65, :], lhsT=ztuN[:, lkt, P, hsl[h][0]:hsl[h][1]],
                rhs=rhs,
                start=(nem[h] == 0), stop=(nem[h] == ncontrib - 1))
            nem[h] += 1

        def emit_fills(upto):
            nonlocal nfe
            while nfe < upto:
                a, h = fills[nfe]
                off = (J - (a >> 2) - 1) * NQB
                av_mm(h, a, tt_tiles[(a, h)][:, off:off + NQB])
                nfe += 1

        # AV for strip g-1 is emitted after strip g's QK so the in-order PE
        # queue never waits on exp; deferred items drain in pairs after odd
        # strips to keep QK's sc double-buffer slot parity.
        for g in range(nstr):
            b = b0 + g
            sc = sc_pool.tile([128, 2 * NQB], F32, tag="sc")
            for h in range(2):
                r0 = 64 * h
                nc.tensor.matmul(
                    sc[:, h * NQB:(h + 1) * NQB],
                    lhsT=ztuT[r0:r0 + 64, P, b * NKT:(b + 1) * NKT],
                    rhs=ztuT[r0:r0 + 64, P, q0:q0 + NQB],
                    start=True, stop=True, tile_position=(r0, 0))
            nc.scalar.activation(
                out=ex_all[:, b, :, :].rearrange("p h q -> p (h q)"), in_=sc,
                func=mybir.ActivationFunctionType.Exp, scale=SCALE)
            if g >= 1:
                for h in range(2):
                    av_mm(h, b - 1, ex_all[:, b - 1, h, :])
            if g % 2 == 1:
                drain(2)
            if fills and g >= 2:
                emit_fills(len(fills) if g == nstr - 1
                           else -(-len(fills) * (g - 1) // (nstr - 2)))
            if SYM and J < 3:
                # bounce computed strips for later qbs (ACT ring), in two
                # chunks so early readers aren't gated on the full write
                if b == 11 and b0 + 4 <= 11:
                    for h in range(2):
                        bn = bn_pool.tile([12, 128, NQB], BF, tag=f"bn{h}{J}",
                                          name=f"bn{h}{J}")
                        bn_tiles[(h, J)] = bn
                        nc.scalar.dma_start(
                            out=bn[0:8 - 4 * J].rearrange("s j q -> j s q"),
                            in_=ex_all[:, b0 + 4:12, h, :])
                if b == 15:
                    for h in range(2):
                        if (h, J) not in bn_tiles:
                            bn = bn_pool.tile([12, 128, NQB], BF,
                                              tag=f"bn{h}{J}", name=f"bn{h}{J}")
                            bn_tiles[(h, J)] = bn
                        nc.scalar.dma_start(
                            out=bn_tiles[(h, J)][8 - 4 * J:12 - 4 * J]
                            .rearrange("s j q -> j s q"),
                            in_=ex_all[:, 12:16, h, :])
        for h in range(2):
            av_mm(h, NTB - 1, ex_all[:, NTB - 1, h, :])
        if fills:
            emit_fills(len(fills))
        if SYM and J < 3:
            emit_reads(P, J)

        # ---- softmax normalize + proj2, deferred into the next qb's
        # instruction stream so the in-order PE queue never waits on the
        # DVE normalization chain ----
        nc.vector.tensor_copy(out=dn[0:1, :], in_=avs[0][64:65, :])
        nc.vector.tensor_copy(out=dn[32:33, :], in_=avs[1][64:65, :])
        sn = sn_pool.tile([128, NQB], BF)

        def norm_item(avs=avs, sn=sn):
            bc = sc_pool.tile([128, 2 * NQB], F32, tag="sc", name="bc")
            bcv = bc[:, 0:NQB]
            nc.tensor.matmul(bcv, lhsT=sel, rhs=dn, start=True, stop=True)
            rcv = rc_pool.tile([128, NQB], F32)
            nc.vector.reciprocal_approx_fast(out=rcv, in_=bcv)
            nc.vector.tensor_tensor(
                out=sn[0:64, :], in0=avs[0][0:DH, :], in1=rcv[0:64, :],
                op=mybir.AluOpType.mult)
            nc.vector.tensor_tensor(
                out=sn[64:128, :], in0=avs[1][0:DH, :], in1=rcv[64:128, :],
                op=mybir.AluOpType.mult)

        def p2_item(tq, ch, P=P, q0=q0, sn=sn):
            p2 = sc_pool.tile([128, 2 * NQB], F32, tag="sc", name="p2")
            p2v = p2[:, 0:NQB]
            nc.tensor.matmul(
                p2v, lhsT=sn[:, tq * 128:(tq + 1) * 128],
                rhs=wg_sb[:, P, ch * NQB:(ch + 1) * NQB],
                start=True, stop=True)
            p2s = p2s_pool.tile([128, NQB], BF, tag="p2s")
            nc.vector.tensor_copy(out=p2s, in_=p2v)
            r0 = q0 + tq * 128
            nc.gpsimd.dma_start(
                out=outp[P, r0:r0 + 128, ch * NQB:(ch + 1) * NQB], in_=p2s)

        pending.append(norm_item)
        for tq in range(4):
            for ch in range(2):
                pending.append(lambda tq=tq, ch=ch: p2_item(tq, ch))

    for P in range(NPAIR):
        for J in range(4):
            do_qb(P, J)
        bn_tiles.clear()
        tt_tiles.clear()
    drain(len(pending))


def _get_nc():
    if "nc" not in _CACHE:
        _CACHE["nc"] = _build_kernel()
    return _CACHE["nc"]


def make_in_maps(ZT, W):
    ZT = np.asarray(ZT, dtype=np.float32)
    W = np.asarray(W, dtype=np.float32)
    ztt = [np.ascontiguousarray(ZT[b].T).astype(ml_dtypes.bfloat16)
           for b in range(B)]
    wghalf = []
    for g in range(2):
        wgf = W[g * KP:(g + 1) * KP, :]
        wghalf.append((np.ascontiguousarray(wgf.T).astype(ml_dtypes.bfloat16),
                       np.ascontiguousarray(wgf).astype(ml_dtypes.bfloat16)))
    in_maps = []
    for c in range(8):
        b, g = c // 2, c % 2
        in_maps.append({"ztt": ztt[b], "wgt": wghalf[g][0], "wg": wghalf[g][1]})
    return in_maps


def kernel(ZT: np.ndarray, W: np.ndarray) -> np.ndarray:
    nc = _get_nc()
    res = run_bass_kernel_spmd(nc, make_in_maps(ZT, W), core_ids=list(range(8)))
    out = np.zeros((B, N, C), dtype=np.float32)
    for c in range(8):
        out[c // 2] += res.results[c]["outp"].astype(np.float32).sum(axis=0)
    return out


if __name__ == "__main__":
    rng = np.random.default_rng(0)
    zt = rng.standard_normal((B, N, C), dtype=np.float32)
    w = rng.standard_normal((C, C), dtype=np.float32) * C ** -0.5
    o = kernel(zt, w)
    print("out", o.shape, o.dtype, float(np.abs(o).mean()))
